# revision 1
# baseline (speedup 1.0000x reference)
"""Trainium2 Bass kernel for ExternalEmbeddingSelfAttention.

Computation (per batch b):
    q     = hs @ Wq + bq           [S,H]
    k_tok = hs @ Wk + bk           [S,H]
    v_tok = hs @ Wv + bv           [S,H]
    k_ext = ext @ Wk + bk          [E,H]
    v_ext = ext @ Wv + bv          [E,H]
    s_self[t] = q[t] . k_tok[t]                (per-token self score)
    s_ext = q @ k_ext^T            [S,E]
    probs = softmax([s_self, s_ext], axis=-1)  (no 1/sqrt(d) scaling)
    out   = probs[:,0:1]*v_tok + probs[:,1:] @ v_ext

Sharding: 8 cores, each takes 1024 contiguous tokens of the flattened
(B*S, H) token axis (core i -> batch i//2, S-half i%2).  Each core also
computes its batch's external projections (duplicated across the 2 cores
sharing a batch; minor cost).

Device algorithm (per core, T=1024 tokens):
  - Host pre-transposes the hs/ext shards (xT=[H,T], eT=[H,E]) so every
    matmul operand is already in lhsT/rhs layout.
  - QT = Wq^T @ xT (+bq)        hidden-major [H, T]  (f32 for score precision)
  - KxT = Wk^T @ eT (+bk)       hidden-major [H, E]  (f32)
  - s_self accumulated row-oriented [2, T] on PE via ones^T @ (K_tok^T*QT)
    (single PSUM accumulation group; start=True clears a whole PSUM bank,
    so per-column groups in one bank are illegal), then transposed to
    per-token columns.
  - Vx = ext @ Wv (+bv)         E-major [E, H]  (fp32r)
  - Attention per 128-token tile: s_ext = QT.T @ KxT (true fp32 matmul,
    4-pass); softmax with the self column folded in; unnormalized probs
    are PE-transposed; ctx PSUM accumulates BOTH the ext context
    (Pt.T @ Vx) and the self term ((xT * p_self_bcast).T @ Wv + p_self x bv)
    in one group; the final PSUM->SBUF copy applies the 1/Z scale.

All V-side matmuls run as float32r (FP22 multiply, fp32 accumulate) at
full PE rate; the score pipeline (s_ext) runs true fp32.
"""

import numpy as np

B, S, E, H = 4, 2048, 512, 1024
NCORES = 8
T = (B * S) // NCORES  # tokens per core = 1024

_RUNNER_CACHE = {}
_WSTREAM_BUFS = 2

_ONESL = np.ones((1, 128), dtype=np.float32)
_ONESC = np.ones((128, 2), dtype=np.float32)
_EYE = np.eye(128, dtype=np.float32)


# --------------------------------------------------------------------------
# device kernel emission
# --------------------------------------------------------------------------

def _emit(nc, tc, ctx, T, H, E, reps=1):
    import contextlib
    import concourse.mybir as mybir

    f32 = mybir.dt.float32
    f32r = mybir.dt.float32r
    Alu = mybir.AluOpType
    Act = mybir.ActivationFunctionType
    X = mybir.AxisListType.X

    KC = H // 128          # contraction chunks over h_in; also h_out tiles
    NT = T // 128          # token tiles
    NE = E // 128          # ext tiles
    WH = min(512, H)       # h_out free-dim chunk
    NH = H // WH
    WT = min(512, T)       # token free-dim chunk
    NTW = T // WT
    assert E <= 512, "s_ext PSUM group assumes E <= 512"

    xT_d = nc.declare_dram_parameter("xT", [H, T], f32, isOutput=False)
    eT_d = nc.declare_dram_parameter("eT", [H, E], f32, isOutput=False)
    wq_d = nc.declare_dram_parameter("Wq", [H, H], f32, isOutput=False)
    wk_d = nc.declare_dram_parameter("Wk", [H, H], f32, isOutput=False)
    wv_d = nc.declare_dram_parameter("Wv", [H, H], f32, isOutput=False)
    bqc_d = nc.declare_dram_parameter("bqc", [128, KC], f32, isOutput=False)
    bkc_d = nc.declare_dram_parameter("bkc", [128, KC], f32, isOutput=False)
    bvr_d = nc.declare_dram_parameter("bvr", [1, H], f32, isOutput=False)
    onesl_d = nc.declare_dram_parameter("onesl", [1, 128], f32, isOutput=False)
    onesc_d = nc.declare_dram_parameter("onesc", [128, 2], f32, isOutput=False)
    eye_d = nc.declare_dram_parameter("eye", [128, 128], f32, isOutput=False)
    out_d = nc.declare_dram_parameter("out", [T, H], f32, isOutput=True)

    cp = ctx.enter_context(tc.tile_pool(name="cp", bufs=1))
    wstream = ctx.enter_context(tc.tile_pool(name="wstream", bufs=_WSTREAM_BUFS))
    small = ctx.enter_context(tc.tile_pool(name="small", bufs=3))

    # ---- persistent SBUF tensors -----------------------------------------
    QT = cp.tile([128, KC * T], f32r, name="QT")    # Q^T hidden-major
    KxT = cp.tile([128, KC * E], f32r, name="KxT")  # K_ext^T hidden-major
    Vx = cp.tile([128, NE * H], f32r, name="Vx")    # V_ext E-major
    xT = cp.tile([128, KC * T], f32r, name="xT")    # chunk kc: cols [kc*T,+T)
    Wv = cp.tile([128, KC * H], f32r, name="Wv")
    ss_col = cp.tile([128, 2 * NT], f32, name="ss_col")
    ss_row = cp.tile([2, T], f32, name="ss_row")
    bqc = cp.tile([128, KC], f32, name="bqc")
    bkc = cp.tile([128, KC], f32, name="bkc")
    bvr = cp.tile([1, H], f32r, name="bvr")
    ones_l = cp.tile([1, 128], f32r, name="ones_l")
    ones_c = cp.tile([128, 2], f32r, name="ones_c")
    ident = cp.tile([128, 128], f32r, name="ident")

    for kc in range(KC):
        nc.sync.dma_start(xT[:, kc * T:(kc + 1) * T],
                          xT_d[kc * 128:(kc + 1) * 128, :].bitcast(f32r))
    nc.sync.dma_start(bqc[:], bqc_d[:])
    nc.sync.dma_start(bkc[:], bkc_d[:])
    nc.sync.dma_start(bvr[:], bvr_d[:].bitcast(f32r))
    nc.sync.dma_start(ones_l[:], onesl_d[:].bitcast(f32r))
    nc.sync.dma_start(ones_c[:], onesc_d[:].bitcast(f32r))
    nc.sync.dma_start(ident[:], eye_d[:].bitcast(f32r))
    # bv broadcast to all 128 partitions (the value bias is shared by every
    # attended value and probs sum to 1, so it adds once at the end)
    bvb = cp.tile([128, H], f32, name="bvb")
    with tc.tile_pool(name="ps_bv", bufs=2, space="PSUM") as ps_bv:
        for n in range(NH):
            pbv = ps_bv.tile([128, WH], f32, name="pbv")
            nc.tensor.matmul(pbv[:], ones_l[:], bvr[:, n * WH:(n + 1) * WH],
                             start=True, stop=True)
            nc.vector.tensor_copy(bvb[:, n * WH:(n + 1) * WH], pbv[:])
    for kc in range(KC):
        nc.scalar.dma_start(Wv[:, kc * H:(kc + 1) * H],
                            wv_d[kc * 128:(kc + 1) * 128, :].bitcast(f32r))

    loop_cm = tc.For_i(0, reps, 1) if reps > 1 else contextlib.nullcontext()
    with loop_cm:
        # ---- phase Q: QT = Wq^T @ xT (+bq) -------------------------------
        with tc.tile_pool(name="ps_q", bufs=4, space="PSUM") as ps_q:
            wq_r = wq_d.rearrange("(kc p) h -> p kc h", p=128)
            for ho in range(KC):
                psq = [ps_q.tile([128, WT], f32, name="psq")
                       for _ in range(NTW)]
                w = wstream.tile([128, KC * 128], f32r, name="w")
                nc.sync.dma_start(
                    w[:].rearrange("p (kc c) -> p kc c", c=128),
                    wq_r[:, :, ho * 128:(ho + 1) * 128].bitcast(f32r))
                for kc in range(KC):
                    for n in range(NTW):
                        nc.tensor.matmul(
                            psq[n][:], w[:, kc * 128:(kc + 1) * 128],
                            xT[:, kc * T + n * WT: kc * T + (n + 1) * WT],
                            start=(kc == 0), stop=(kc == KC - 1))
                for n in range(NTW):
                    nc.vector.tensor_scalar_add(
                        QT[:, ho * T + n * WT: ho * T + (n + 1) * WT],
                        psq[n][:], bqc[:, ho:ho + 1])

        # ---- phase K + s_self, and V_ext (uses eT) -----------------------
        with tc.tile_pool(name="cp_e", bufs=1) as cp_e:
            eT = cp_e.tile([128, KC * E], f32r, name="eT")
            for kc in range(KC):
                nc.scalar.dma_start(
                    eT[:, kc * E:(kc + 1) * E],
                    eT_d[kc * 128:(kc + 1) * 128, :].bitcast(f32r))

            with tc.tile_pool(name="ps_ke", bufs=2, space="PSUM") as ps_ke, \
                 tc.tile_pool(name="ps_kt", bufs=4, space="PSUM") as ps_kt, \
                 tc.tile_pool(name="ps_ss", bufs=1, space="PSUM") as ps_ss, \
                 tc.tile_pool(name="work_k", bufs=2) as work_k:
                sself = ps_ss.tile([2, T], f32, name="sself")
                wk_r = wk_d.rearrange("(kc p) h -> p kc h", p=128)
                for ho in range(KC):
                    pske = ps_ke.tile([128, E], f32, name="pske")
                    pskt = [ps_kt.tile([128, WT], f32, name="pskt")
                            for _ in range(NTW)]
                    w = wstream.tile([128, KC * 128], f32r, name="w")
                    nc.sync.dma_start(
                        w[:].rearrange("p (kc c) -> p kc c", c=128),
                        wk_r[:, :, ho * 128:(ho + 1) * 128].bitcast(f32r))
                    for kc in range(KC):
                        wc = w[:, kc * 128:(kc + 1) * 128]
                        nc.tensor.matmul(pske[:], wc,
                                         eT[:, kc * E:(kc + 1) * E],
                                         start=(kc == 0), stop=(kc == KC - 1))
                        for n in range(NTW):
                            nc.tensor.matmul(
                                pskt[n][:], wc,
                                xT[:, kc * T + n * WT: kc * T + (n + 1) * WT],
                                start=(kc == 0), stop=(kc == KC - 1))
                    nc.vector.tensor_scalar_add(KxT[:, ho * E:(ho + 1) * E],
                                                pske[:], bkc[:, ho:ho + 1])
                    # d = (k_tok^T + bk) * QT, straight from PSUM
                    d = work_k.tile([128, T], f32r, name="d")
                    for n in range(NTW):
                        nc.vector.scalar_tensor_tensor(
                            d[:, n * WT:(n + 1) * WT], pskt[n][:],
                            bkc[:, ho:ho + 1],
                            QT[:, ho * T + n * WT: ho * T + (n + 1) * WT],
                            Alu.add, Alu.mult)
                    for n in range(NTW):
                        nc.tensor.matmul(sself[:, n * WT:(n + 1) * WT],
                                         ones_c[:], d[:, n * WT:(n + 1) * WT],
                                         start=(ho == 0), stop=(ho == KC - 1))
                nc.vector.tensor_copy(ss_row[:], sself[:])

            # s_self [2, T] -> per-token columns ss_col[:, 2m]
            with tc.tile_pool(name="ps_tr0", bufs=2, space="PSUM") as ps_tr0:
                for m in range(NT):
                    pst2 = ps_tr0.tile([128, 2], f32, name="pst2")
                    nc.tensor.transpose(pst2[:],
                                        ss_row[:, m * 128:(m + 1) * 128],
                                        ident[0:2, 0:2].bitcast(f32))
                    nc.vector.tensor_copy(ss_col[:, 2 * m:2 * m + 2], pst2[:])

            # ---- phase V_ext: Vx = (eT)^T @ Wv (+bv), E-major ------------
            with tc.tile_pool(name="ps_ve", bufs=4, space="PSUM") as ps_ve:
                for eo in range(NE):
                    psv = [ps_ve.tile([128, WH], f32, name="psv")
                           for _ in range(NH)]
                    for kc in range(KC):
                        lhsT = eT[:, kc * E + eo * 128: kc * E + (eo + 1) * 128]
                        for n in range(NH):
                            nc.tensor.matmul(
                                psv[n][:], lhsT,
                                Wv[:, kc * H + n * WH: kc * H + (n + 1) * WH],
                                start=(kc == 0), stop=(kc == KC - 1))
                    for n in range(NH):
                        nc.scalar.copy(
                            Vx[:, eo * H + n * WH: eo * H + (n + 1) * WH],
                            psv[n][:])

        # ---- attention per token tile ------------------------------------
        with tc.tile_pool(name="ps_s", bufs=2, space="PSUM") as ps_att, \
             tc.tile_pool(name="ps_tr", bufs=2, space="PSUM") as ps_tr, \
             tc.tile_pool(name="ps_c", bufs=2, space="PSUM") as ps_c, \
             tc.tile_pool(name="work_a", bufs=2) as work_a, \
             tc.tile_pool(name="work_s", bufs=3) as work_s:
            for m in range(NT):
                # s_ext = Q^T.T @ K_ext^T  -> [128 tokens, E]  (true fp32)
                ps_s = ps_att.tile([128, E], f32, name="ps_s")
                for kc in range(KC):
                    nc.tensor.matmul(
                        ps_s[:],
                        QT[:, kc * T + m * 128: kc * T + (m + 1) * 128],
                        KxT[:, kc * E:(kc + 1) * E],
                        start=(kc == 0), stop=(kc == KC - 1))

                nmx = small.tile([128, 1], f32, name="nmx")
                nc.vector.tensor_reduce(nmx[:], ps_s[:], axis=X, op=Alu.max,
                                        negate=True)
                nself = small.tile([128, 1], f32, name="nself")
                nc.vector.tensor_scalar_mul(nself[:],
                                            ss_col[:, 2 * m:2 * m + 1], -1.0)
                nmx2 = small.tile([128, 1], f32, name="nmx2")
                nc.vector.tensor_tensor(nmx2[:], nmx[:], nself[:], Alu.min)

                # unnormalized probs; Z accumulated on the fly
                pe = work_a.tile([128, E], f32r, name="pe")
                Ze = small.tile([128, 1], f32, name="Ze")
                nc.scalar.activation(pe[:], ps_s[:], Act.Exp, bias=nmx2[:],
                                     scale=1.0, accum_out=Ze[:])
                p_self = small.tile([128, 1], f32, name="p_self")
                nc.scalar.activation(p_self[:], ss_col[:, 2 * m:2 * m + 1],
                                     Act.Exp, bias=nmx2[:], scale=1.0)
                Zt = small.tile([128, 1], f32, name="Zt")
                nc.vector.tensor_tensor(Zt[:], Ze[:], p_self[:], Alu.add)
                r = small.tile([128, 1], f32, name="r")
                nc.vector.reciprocal(r[:], Zt[:])

                # p_self as a row + broadcast to all partitions (for the
                # per-column scaling of the v_tok lhsT)
                psr_ps = ps_tr.tile([1, 128], f32, name="psr_ps", bufs=1)
                nc.tensor.transpose(psr_ps[:], p_self[:].bitcast(f32),
                                    ident[:].bitcast(f32))
                psr = work_s.tile([1, 128], f32r, name="psr")
                nc.vector.tensor_copy(psr[:], psr_ps[:])
                bc_ps = ps_tr.tile([128, 128], f32, name="bc_ps", bufs=1)
                nc.tensor.matmul(bc_ps[:], ones_l[:], psr[:],
                                 start=True, stop=True)
                Bc = work_s.tile([128, 128], f32r, name="Bc")
                nc.vector.tensor_copy(Bc[:], bc_ps[:])

                # transpose unnormalized ext probs -> Pt (E-major)
                Pt = work_a.tile([128, NE * 128], f32r, name="Pt")
                for ec in range(NE):
                    pst = ps_tr.tile([128, 128], f32r, name="pst")
                    nc.tensor.transpose(pst[:],
                                        pe[:, ec * 128:(ec + 1) * 128],
                                        ident[:])
                    nc.vector.tensor_copy(Pt[:, ec * 128:(ec + 1) * 128],
                                          pst[:])

                # ctx = Pt.T @ Vx + (xT*p_self).T @ Wv + p_self x bv,
                # all in one PSUM accumulation group per 512-col chunk
                psc = [ps_c.tile([128, WH], f32, name="psc")
                       for _ in range(NH)]
                for ec in range(NE):
                    lhsT = Pt[:, ec * 128:(ec + 1) * 128]
                    for n in range(NH):
                        nc.tensor.matmul(
                            psc[n][:], lhsT,
                            Vx[:, ec * H + n * WH: ec * H + (n + 1) * WH],
                            start=(ec == 0), stop=False)
                for kc in range(KC):
                    xs = work_s.tile([128, 128], f32r, name="xs")
                    nc.vector.tensor_tensor(
                        xs[:], xT[:, kc * T + m * 128: kc * T + (m + 1) * 128],
                        Bc[:], Alu.mult)
                    for n in range(NH):
                        nc.tensor.matmul(
                            psc[n][:], xs[:],
                            Wv[:, kc * H + n * WH: kc * H + (n + 1) * WH],
                            start=False, stop=(kc == KC - 1))

                # normalize on the way out
                out_sb = work_a.tile([128, H], f32, name="out_sb")
                for n in range(NH):
                    nc.vector.scalar_tensor_tensor(
                        out_sb[:, n * WH:(n + 1) * WH], psc[n][:], r[:],
                        bvb[:, n * WH:(n + 1) * WH], Alu.mult, Alu.add)
                nc.scalar.dma_start(out_d[m * 128:(m + 1) * 128, :], out_sb[:])


def _build_module(T, H, E, reps=1):
    from contextlib import ExitStack
    import concourse.tile as tile
    from concourse import bacc

    nc = bacc.Bacc(None)
    with ExitStack() as ctx:
        tc = ctx.enter_context(tile.TileContext(nc))
        _emit(nc, tc, ctx, T, H, E, reps)
    nc.finalize()
    return nc


# --------------------------------------------------------------------------
# host side
# --------------------------------------------------------------------------

def _shard_inputs(hidden_states, external_embeddings, Wq, bq, Wk, bk, Wv, bv):
    """Build the per-core input maps (host-side layout prep)."""
    hs = np.asarray(hidden_states, dtype=np.float32)
    ext = np.asarray(external_embeddings, dtype=np.float32)
    Wq = np.ascontiguousarray(np.asarray(Wq, dtype=np.float32))
    Wk = np.ascontiguousarray(np.asarray(Wk, dtype=np.float32))
    Wv = np.ascontiguousarray(np.asarray(Wv, dtype=np.float32))
    bq = np.asarray(bq, dtype=np.float32)
    bk = np.asarray(bk, dtype=np.float32)
    bv = np.asarray(bv, dtype=np.float32)

    KC = H // 128
    bqc = np.ascontiguousarray(bq.reshape(KC, 128).T)  # [128, KC]
    bkc = np.ascontiguousarray(bk.reshape(KC, 128).T)
    bvr = np.ascontiguousarray(bv.reshape(1, H))

    flat = hs.reshape(B * S, H)
    in_maps = []
    for c in range(NCORES):
        b = (c * T) // S
        xT = np.ascontiguousarray(flat[c * T:(c + 1) * T, :].T)  # [H, T]
        eT = np.ascontiguousarray(ext[b].T)                      # [H, E]
        in_maps.append({
            "xT": xT, "eT": eT,
            "Wq": Wq, "Wk": Wk, "Wv": Wv,
            "bqc": bqc, "bkc": bkc, "bvr": bvr,
            "onesl": _ONESL, "onesc": _ONESC, "eye": _EYE,
        })
    return in_maps


def kernel(hidden_states, external_embeddings, Wq, bq, Wk, bk, Wv, bv):
    from concourse.bass_utils import run_bass_kernel_spmd

    key = "main"
    if key not in _RUNNER_CACHE:
        _RUNNER_CACHE[key] = _build_module(T, H, E)
    nc = _RUNNER_CACHE[key]

    in_maps = _shard_inputs(hidden_states, external_embeddings,
                            Wq, bq, Wk, bk, Wv, bv)
    res = run_bass_kernel_spmd(nc, in_maps, list(range(NCORES)))
    out = np.concatenate([res.results[c]["out"] for c in range(NCORES)],
                         axis=0)
    return out.reshape(B, S, H)



# revision 18
# speedup vs baseline: 1.6591x; 1.6591x over previous
"""Trainium2 Bass kernel for ExternalEmbeddingSelfAttention.

Computation (per batch b):
    q     = hs @ Wq + bq           [S,H]
    k_tok = hs @ Wk + bk           [S,H]
    v_tok = hs @ Wv + bv           [S,H]
    k_ext = ext @ Wk + bk          [E,H]
    v_ext = ext @ Wv + bv          [E,H]
    s_self[t] = q[t] . k_tok[t]                (per-token self score)
    s_ext = q @ k_ext^T            [S,E]
    probs = softmax([s_self, s_ext], axis=-1)  (no 1/sqrt(d) scaling)
    out   = probs[:,0:1]*v_tok + probs[:,1:] @ v_ext

Sharding: 8 cores, each takes 1024 contiguous tokens of the flattened
(B*S, H) token axis (core i -> batch i//2, S-half i%2).  Each core also
computes its batch's external projections (duplicated across the 2 cores
sharing a batch; minor cost).

Device algorithm (per core, T=1024 tokens), v3:
  - Everything input-side lives in SBUF persistently: xT=[H,T], eT=[H,E]
    (host pre-transposed), Wq/Wk (f32), Wv (bf16), biases.  The steady
    state loop moves NO weight DMA traffic; only the output streams out.
  - QT = Wq^T @ xT (+bq on ACT)  hidden-major [H, T] f32r
  - KxT = Wk^T @ eT (+bk on ACT) hidden-major [H, E] f32r
  - s_self accumulated row-oriented [2, T] on PE via ones^T @ (K_tok^T*QT)
    (d-mult on DVE straight from PSUM), then PE-transposed to per-token
    columns (interleaved with the V_ext phase).
  - Vx = ext @ Wv (raw, no bias)  E-major [E, H] bf16
  - Attention per 128-token tile:
      s_ext = QT.T @ KxT                       (PE, fp32r)
      vtok  = xT_tile.T @ Wv -> PSUM           (PE, independent of softmax)
      softmax w/ self column folded in         (DVE max / ACT exp -> bf16)
      Pt    = transpose(exp probs)             (PE, bf16: 1 cyc/row)
      ctx   = Pt.T @ Vx -> PSUM                (PE, bf16 + fast weight load)
      out   = psc*r + vtok*(p_self*r) + bvb    (DVE, 2 fused STT passes)
    The vtok matmuls depend only on xT/Wv, so the PE never waits on the
    softmax chain; p_self scaling happens in the PSUM->SBUF combine.

Precision: the score pipeline (Wq, Wk, QT, KxT, d/s_self) is all fp32/
fp32r.  Only the value path (Wv, Vx, exp-probs) is bf16; probs and
values are O(1) so the ~2^-9 relative error lands well inside the 2e-2
gate.
"""

import numpy as np

B, S, E, H = 4, 2048, 512, 1024
NCORES = 8
T = (B * S) // NCORES  # tokens per core = 1024

_RUNNER_CACHE = {}

# experiment knob (probes may flip; default is the shipping configuration)
_LOOP_MODE = "staggered"   # "staggered" | "plain"

_ONESC = np.ones((128, 2), dtype=np.float32)
_EYE = np.eye(128, dtype=np.float32)


# --------------------------------------------------------------------------
# device kernel emission
# --------------------------------------------------------------------------

def _emit(nc, tc, ctx, T, H, E, reps=1):
    import contextlib
    import concourse.mybir as mybir

    f32 = mybir.dt.float32
    f32r = mybir.dt.float32r
    bf16 = mybir.dt.bfloat16
    Alu = mybir.AluOpType
    Act = mybir.ActivationFunctionType
    X = mybir.AxisListType.X

    KC = H // 128          # contraction chunks over h_in; also h_out tiles
    NT = T // 128          # token tiles
    NE = E // 128          # ext tiles
    WH = min(512, H)       # h_out free-dim chunk
    NH = H // WH
    WT = min(512, T)       # token free-dim chunk
    NTW = T // WT
    assert E <= 512, "s_ext PSUM group assumes E <= 512"

    xT_d = nc.declare_dram_parameter("xT", [H, T], f32, isOutput=False)
    eT_d = nc.declare_dram_parameter("eT", [H, E], f32, isOutput=False)
    wq_d = nc.declare_dram_parameter("Wq", [H, H], f32, isOutput=False)
    wk_d = nc.declare_dram_parameter("Wk", [H, H], f32, isOutput=False)
    wv_d = nc.declare_dram_parameter("Wv", [H, H], f32, isOutput=False)
    bqc_d = nc.declare_dram_parameter("bqc", [128, KC], f32, isOutput=False)
    bkc_d = nc.declare_dram_parameter("bkc", [128, KC], f32, isOutput=False)
    bvb_d = nc.declare_dram_parameter("bvb", [128, H], f32, isOutput=False)
    onesc_d = nc.declare_dram_parameter("onesc", [128, 2], f32, isOutput=False)
    eye_d = nc.declare_dram_parameter("eye", [128, 128], f32, isOutput=False)
    eyeb_d = nc.declare_dram_parameter("eyeb", [128, 128], bf16,
                                       isOutput=False)
    out_d = nc.declare_dram_parameter("out", [T, H], f32, isOutput=True)

    cp = ctx.enter_context(tc.tile_pool(name="cp", bufs=1))
    wstream = ctx.enter_context(tc.tile_pool(name="wstream", bufs=2))
    small = ctx.enter_context(tc.tile_pool(name="small", bufs=3))

    # ---- persistent SBUF tensors -----------------------------------------
    QT = cp.tile([128, KC * T], f32r, name="QT")    # Q^T hidden-major
    KxT = cp.tile([128, KC * E], f32r, name="KxT")  # K_ext^T hidden-major
    Vx = cp.tile([128, NE * H], bf16, name="Vx")    # V_ext E-major (raw)
    xT = cp.tile([128, KC * T], f32r, name="xT")    # chunk kc: cols [kc*T,+T)
    eT = cp.tile([128, KC * E], f32r, name="eT")    # chunk kc: cols [kc*E,+E)
    Wq = cp.tile([128, KC * H], f32r, name="Wq")    # chunk kc: cols [kc*H,+H)
    Wv = cp.tile([128, KC * H], f32r, name="Wv")
    ss_col = cp.tile([128, 2 * NT], f32, name="ss_col")
    ss_row = cp.tile([2, T], f32, name="ss_row")
    bqc = cp.tile([128, KC], f32, name="bqc")
    bkc = cp.tile([128, KC], f32, name="bkc")
    bvb = cp.tile([128, H], f32, name="bvb")
    ones_c = cp.tile([128, 2], f32r, name="ones_c")
    ident = cp.tile([128, 128], f32r, name="ident")
    identb = cp.tile([128, 128], bf16, name="identb")

    for kc in range(KC):
        nc.sync.dma_start(xT[:, kc * T:(kc + 1) * T],
                          xT_d[kc * 128:(kc + 1) * 128, :].bitcast(f32r))
        nc.sync.dma_start(eT[:, kc * E:(kc + 1) * E],
                          eT_d[kc * 128:(kc + 1) * 128, :].bitcast(f32r))
        nc.sync.dma_start(Wq[:, kc * H:(kc + 1) * H],
                          wq_d[kc * 128:(kc + 1) * 128, :].bitcast(f32r))
        nc.scalar.dma_start(Wv[:, kc * H:(kc + 1) * H],
                            wv_d[kc * 128:(kc + 1) * 128, :].bitcast(f32r))
    nc.sync.dma_start(bqc[:], bqc_d[:])
    nc.sync.dma_start(bkc[:], bkc_d[:])
    nc.sync.dma_start(bvb[:], bvb_d[:])
    nc.sync.dma_start(ones_c[:], onesc_d[:].bitcast(f32r))
    nc.sync.dma_start(ident[:], eye_d[:].bitcast(f32r))
    nc.sync.dma_start(identb[:], eyeb_d[:])

    Eng = mybir.EngineType
    if reps <= 1:
        loop_cm = contextlib.nullcontext()
    elif _LOOP_MODE == "plain":
        loop_cm = tc.For_i(0, reps, 1)
    else:
        loop_cm = tc.For_i(0, reps, 1,
                           hint_engines=(Eng.PE, Eng.DVE, Eng.Activation,
                                         Eng.SP, Eng.Pool),
                           staggered_reset=True)
    with loop_cm:
        # ---- phase Q: QT = Wq^T @ xT (+bq) -------------------------------
        with tc.tile_pool(name="ps_q", bufs=4, space="PSUM") as ps_q:
            for ho in range(KC):
                psq = [ps_q.tile([128, WT], f32, name="psq")
                       for _ in range(NTW)]
                for kc in range(KC):
                    w = Wq[:, kc * H + ho * 128: kc * H + (ho + 1) * 128]
                    for n in range(NTW):
                        nc.tensor.matmul(
                            psq[n][:], w,
                            xT[:, kc * T + n * WT: kc * T + (n + 1) * WT],
                            start=(kc == 0), stop=(kc == KC - 1))
                for n in range(NTW):
                    nc.scalar.activation(
                        QT[:, ho * T + n * WT: ho * T + (n + 1) * WT],
                        psq[n][:], Act.Identity, bias=bqc[:, ho:ho + 1])

        # ---- phase K: KxT (+bk), K_tok -> s_self -------------------------
        with tc.tile_pool(name="ps_ke", bufs=2, space="PSUM") as ps_ke, \
             tc.tile_pool(name="ps_kt", bufs=4, space="PSUM") as ps_kt, \
             tc.tile_pool(name="ps_ss", bufs=1, space="PSUM") as ps_ss, \
             tc.tile_pool(name="work_k", bufs=2) as work_k:
            sself = ps_ss.tile([2, T], f32, name="sself")
            wk_r = wk_d.rearrange("(kc p) h -> p kc h", p=128)
            for ho in range(KC):
                pske = ps_ke.tile([128, E], f32, name="pske")
                pskt = [ps_kt.tile([128, WT], f32, name="pskt")
                        for _ in range(NTW)]
                w = wstream.tile([128, KC * 128], f32r, name="w")
                nc.sync.dma_start(
                    w[:].rearrange("p (kc c) -> p kc c", c=128),
                    wk_r[:, :, ho * 128:(ho + 1) * 128].bitcast(f32r))
                for kc in range(KC):
                    wc = w[:, kc * 128:(kc + 1) * 128]
                    nc.tensor.matmul(pske[:], wc,
                                     eT[:, kc * E:(kc + 1) * E],
                                     start=(kc == 0), stop=(kc == KC - 1))
                    for n in range(NTW):
                        nc.tensor.matmul(
                            pskt[n][:], wc,
                            xT[:, kc * T + n * WT: kc * T + (n + 1) * WT],
                            start=(kc == 0), stop=(kc == KC - 1))
                nc.scalar.activation(KxT[:, ho * E:(ho + 1) * E],
                                     pske[:], Act.Identity,
                                     bias=bkc[:, ho:ho + 1])
                # d = (k_tok^T + bk) * QT, straight from PSUM
                d = work_k.tile([128, T], f32r, name="d")
                for n in range(NTW):
                    nc.vector.scalar_tensor_tensor(
                        d[:, n * WT:(n + 1) * WT], pskt[n][:],
                        bkc[:, ho:ho + 1],
                        QT[:, ho * T + n * WT: ho * T + (n + 1) * WT],
                        Alu.add, Alu.mult)
                for n in range(NTW):
                    nc.tensor.matmul(sself[:, n * WT:(n + 1) * WT],
                                     ones_c[:], d[:, n * WT:(n + 1) * WT],
                                     start=(ho == 0), stop=(ho == KC - 1))
            # split the 2-partition copy across DVE+ACT (slow: 2 lanes only)
            nc.vector.tensor_copy(ss_row[:, 0:WT], sself[:, 0:WT])
            nc.scalar.copy(ss_row[:, WT:T], sself[:, WT:T])

        # ---- phase V_ext: Vx = (eT)^T @ Wv (raw), E-major ----------------
        # (the s_self transposes are interleaved so their DVE/ACT inputs
        # overlap the V_ext matmul stream instead of blocking the PE)
        with tc.tile_pool(name="ps_ve", bufs=4, space="PSUM") as ps_ve, \
             tc.tile_pool(name="ps_tr0", bufs=2, space="PSUM") as ps_tr0:
            for eo in range(NE):
                psv = [ps_ve.tile([128, WH], f32, name="psv")
                       for _ in range(NH)]
                for kc in range(KC):
                    lhsT = eT[:, kc * E + eo * 128: kc * E + (eo + 1) * 128]
                    for n in range(NH):
                        nc.tensor.matmul(
                            psv[n][:], lhsT,
                            Wv[:, kc * H + n * WH: kc * H + (n + 1) * WH],
                            start=(kc == 0), stop=(kc == KC - 1))
                for n in range(NH):
                    nc.scalar.copy(
                        Vx[:, eo * H + n * WH: eo * H + (n + 1) * WH],
                        psv[n][:])
                # two s_self transposes per eo tile
                for m in range(2 * eo, 2 * eo + 2):
                    pst2 = ps_tr0.tile([128, 2], f32, name="pst2")
                    nc.tensor.transpose(pst2[:],
                                        ss_row[:, m * 128:(m + 1) * 128],
                                        ident[0:2, 0:2].bitcast(f32))
                    nc.scalar.copy(ss_col[:, 2 * m:2 * m + 2], pst2[:])

        # ---- attention per token tile ------------------------------------
        with tc.tile_pool(name="ps_s", bufs=2, space="PSUM") as ps_att, \
             tc.tile_pool(name="ps_v", bufs=2, space="PSUM") as ps_v, \
             tc.tile_pool(name="ps_c", bufs=2, space="PSUM") as ps_c, \
             tc.tile_pool(name="ps_tr", bufs=2, space="PSUM") as ps_tr, \
             tc.tile_pool(name="work_a", bufs=2) as work_a, \
             tc.tile_pool(name="work_o", bufs=1) as work_o:
            for m in range(NT):
                # s_ext = Q^T.T @ K_ext^T  -> [128 tokens, E]
                ps_s = ps_att.tile([128, E], f32, name="ps_s")
                for kc in range(KC):
                    nc.tensor.matmul(
                        ps_s[:],
                        QT[:, kc * T + m * 128: kc * T + (m + 1) * 128],
                        KxT[:, kc * E:(kc + 1) * E],
                        start=(kc == 0), stop=(kc == KC - 1))

                # vtok = xT_tile.T @ Wv -> PSUM (raw v_tok, no softmax dep)
                psvt = [ps_v.tile([128, WH], f32, name="psvt")
                        for _ in range(NH)]
                for kc in range(KC):
                    lhsT = xT[:, kc * T + m * 128: kc * T + (m + 1) * 128]
                    for n in range(NH):
                        nc.tensor.matmul(
                            psvt[n][:], lhsT,
                            Wv[:, kc * H + n * WH: kc * H + (n + 1) * WH],
                            start=(kc == 0), stop=(kc == KC - 1))

                nmx = small.tile([128, 1], f32, name="nmx")
                nc.vector.tensor_reduce(nmx[:], ps_s[:], axis=X, op=Alu.max,
                                        negate=True)
                nself = small.tile([128, 1], f32, name="nself")
                nc.vector.tensor_scalar_mul(nself[:],
                                            ss_col[:, 2 * m:2 * m + 1], -1.0)
                nmx2 = small.tile([128, 1], f32, name="nmx2")
                nc.vector.tensor_tensor(nmx2[:], nmx[:], nself[:], Alu.min)

                # unnormalized probs (bf16); Z accumulated on the fly (f32)
                pe = work_a.tile([128, E], bf16, name="pe")
                Ze = small.tile([128, 1], f32, name="Ze")
                nc.scalar.activation(pe[:], ps_s[:], Act.Exp, bias=nmx2[:],
                                     scale=1.0, accum_out=Ze[:])
                p_self = small.tile([128, 1], f32, name="p_self")
                nc.scalar.activation(p_self[:], ss_col[:, 2 * m:2 * m + 1],
                                     Act.Exp, bias=nmx2[:], scale=1.0)
                Zt = small.tile([128, 1], f32, name="Zt")
                nc.vector.tensor_tensor(Zt[:], Ze[:], p_self[:], Alu.add)
                r = small.tile([128, 1], f32, name="r")
                nc.vector.reciprocal(r[:], Zt[:])
                a = small.tile([128, 1], f32, name="a")
                nc.vector.tensor_tensor(a[:], p_self[:], r[:], Alu.mult)

                # transpose unnormalized ext probs -> Pt (E-major, bf16)
                Pt = work_a.tile([128, NE * 128], bf16, name="Pt")
                for ec in range(NE):
                    pst = ps_tr.tile([128, 128], bf16, name="pst")
                    nc.tensor.transpose(pst[:],
                                        pe[:, ec * 128:(ec + 1) * 128],
                                        identb[:])
                    if ec % 2 == 0:
                        nc.scalar.copy(Pt[:, ec * 128:(ec + 1) * 128], pst[:])
                    else:
                        nc.vector.tensor_copy(
                            Pt[:, ec * 128:(ec + 1) * 128], pst[:])

                # ctx_ext = Pt.T @ Vx (unnormalized)
                psc = [ps_c.tile([128, WH], f32, name="psc")
                       for _ in range(NH)]
                for ec in range(NE):
                    lhsT = Pt[:, ec * 128:(ec + 1) * 128]
                    for n in range(NH):
                        nc.tensor.matmul(
                            psc[n][:], lhsT,
                            Vx[:, ec * H + n * WH: ec * H + (n + 1) * WH],
                            start=(ec == 0), stop=(ec == NE - 1))

                # out = psc*r + vtok*(p_self*r) + bvb, two fused STT passes
                out_sb = work_o.tile([128, H], f32, name="out_sb")
                for n in range(NH):
                    nc.vector.scalar_tensor_tensor(
                        out_sb[:, n * WH:(n + 1) * WH], psvt[n][:], a[:],
                        bvb[:, n * WH:(n + 1) * WH], Alu.mult, Alu.add)
                    nc.vector.scalar_tensor_tensor(
                        out_sb[:, n * WH:(n + 1) * WH], psc[n][:], r[:],
                        out_sb[:, n * WH:(n + 1) * WH], Alu.mult, Alu.add)
                nc.scalar.dma_start(out_d[m * 128:(m + 1) * 128, :], out_sb[:])


def _build_module(T, H, E, reps=1):
    from contextlib import ExitStack
    import concourse.tile as tile
    from concourse import bacc

    nc = bacc.Bacc(None)
    with ExitStack() as ctx:
        tc = ctx.enter_context(tile.TileContext(nc))
        _emit(nc, tc, ctx, T, H, E, reps)
    nc.finalize()
    return nc


# --------------------------------------------------------------------------
# host side
# --------------------------------------------------------------------------

def _shard_inputs(hidden_states, external_embeddings, Wq, bq, Wk, bk, Wv, bv):
    """Build the per-core input maps (host-side layout prep)."""
    import ml_dtypes

    hs = np.asarray(hidden_states, dtype=np.float32)
    ext = np.asarray(external_embeddings, dtype=np.float32)
    Wq = np.ascontiguousarray(np.asarray(Wq, dtype=np.float32))
    Wk = np.ascontiguousarray(np.asarray(Wk, dtype=np.float32))
    Wv = np.ascontiguousarray(np.asarray(Wv, dtype=np.float32))
    bq = np.asarray(bq, dtype=np.float32)
    bk = np.asarray(bk, dtype=np.float32)
    bv = np.asarray(bv, dtype=np.float32)

    KC = H // 128
    bqc = np.ascontiguousarray(bq.reshape(KC, 128).T)  # [128, KC]
    bkc = np.ascontiguousarray(bk.reshape(KC, 128).T)
    bvb = np.ascontiguousarray(np.broadcast_to(bv.reshape(1, H), (128, H)))
    eyeb = _EYE.astype(ml_dtypes.bfloat16)

    flat = hs.reshape(B * S, H)
    in_maps = []
    for c in range(NCORES):
        b = (c * T) // S
        xT = np.ascontiguousarray(flat[c * T:(c + 1) * T, :].T)  # [H, T]
        eT = np.ascontiguousarray(ext[b].T)                      # [H, E]
        in_maps.append({
            "xT": xT, "eT": eT,
            "Wq": Wq, "Wk": Wk, "Wv": Wv,
            "bqc": bqc, "bkc": bkc, "bvb": bvb,
            "onesc": _ONESC, "eye": _EYE, "eyeb": eyeb,
        })
    return in_maps


def kernel(hidden_states, external_embeddings, Wq, bq, Wk, bk, Wv, bv):
    from concourse.bass_utils import run_bass_kernel_spmd

    key = "main"
    if key not in _RUNNER_CACHE:
        _RUNNER_CACHE[key] = _build_module(T, H, E)
    nc = _RUNNER_CACHE[key]

    in_maps = _shard_inputs(hidden_states, external_embeddings,
                            Wq, bq, Wk, bk, Wv, bv)
    res = run_bass_kernel_spmd(nc, in_maps, list(range(NCORES)))
    out = np.concatenate([res.results[c]["out"] for c in range(NCORES)],
                         axis=0)
    return out.reshape(B, S, H)


# revision 21
# speedup vs baseline: 1.9634x; 1.1834x over previous
"""Trainium2 Bass kernel for ExternalEmbeddingSelfAttention.

Reference computation (per batch b):
    q     = hs @ Wq + bq           [S,H]
    k_tok = hs @ Wk + bk           [S,H]
    v_tok = hs @ Wv + bv           [S,H]
    k_ext = ext @ Wk + bk          [E,H]
    v_ext = ext @ Wv + bv          [E,H]
    s_self[t] = q[t] . k_tok[t]
    s_ext = q @ k_ext^T            [S,E]
    probs = softmax([s_self, s_ext], axis=-1)  (no 1/sqrt(d) scaling)
    out   = probs[:,0:1]*v_tok + probs[:,1:] @ v_ext

Score reformulation (v6).  Softmax is invariant to adding a constant to a
token's whole score row; q.bk appears in EVERY score of token t (self and
ext), so bk drops out entirely.  What remains factors through the
weight-only matrix C = Wq @ Wk^T and vector w2 = Wk @ bq (both computed
once on the host):
    s_ext[t,e] = x_t.C.ext_e + w2.ext_e   = (xT^T M)[t,e] + u[e]
                 with M = C @ ext^T [H,E],  u = w2^T ext^T [1,E]
    s_self[t]  = x_t.C.x_t + w2.x_t       = sum_h xT[h,t]*y[h,t]
                 with y = C^T x + w2 (w2 folded in as the eviction bias)
This deletes the Q and K_ext projections outright: M costs half a
projection (E=512 vs T=1024 columns) and everything else reuses existing
streams.  Per-core PE work drops ~17% vs the direct formulation.

Sharding: 8 cores, each takes 1024 contiguous tokens of the flattened
(B*S, H) token axis (core i -> batch i//2, S-half i%2).  Each core also
computes its batch's external M/u/V_ext (duplicated across the 2 cores
sharing a batch).

Device algorithm (per core, T=1024 tokens):
  - All operands live in SBUF persistently (xT, eT, C, D=C^T, Wv, biases);
    the steady-state loop has NO weight DMA, only the output stream.
  - y-proj: psy = C^T @ xT per h-tile; d = (psy + w2)*xT on DVE straight
    from PSUM; s_self accumulated [2,T] on PE via ones^T @ d; transposed
    to per-token columns during the V_ext phase.
  - M = D^T @ eT  (hidden-major [H,E], fp32r), u = w2^T @ eT [1,E]
  - Vx = ext @ Wv (raw, no bias)  E-major [E,H] bf16
  - Attention per 128-token tile:
      s_ext = xT_tile^T @ M (+ ones^T x u row)  (PE, fp32r)
      vtok  = xT_tile^T @ Wv -> PSUM            (PE, no softmax dep)
      softmax w/ self column folded in          (DVE max / ACT exp -> bf16)
      Pt    = transpose(exp probs)              (PE, bf16)
      ctx   = Pt^T @ Vx -> PSUM                 (PE, bf16, fast wt load)
      out   = psc*r + vtok*(p_self*r) + bvb     (DVE, 2 fused STT passes)

Precision: the score pipeline is fp32/fp32r end to end (C is formed in
f64 on the host, rounded to f32).  Only the value path (probs, Vx) is
bf16; probs and values are O(1) so the 2^-9 relative error lands well
inside the 2e-2 gate.
"""

import numpy as np

B, S, E, H = 4, 2048, 512, 1024
NCORES = 8
T = (B * S) // NCORES  # tokens per core = 1024

_RUNNER_CACHE = {}

# experiment knob (probes may flip; default is the shipping configuration)
_LOOP_MODE = "staggered"   # "staggered" | "plain"

_ONESC = np.ones((128, 2), dtype=np.float32)
_ONESR = np.ones((1, 128), dtype=np.float32)
_EYE = np.eye(128, dtype=np.float32)


# --------------------------------------------------------------------------
# device kernel emission
# --------------------------------------------------------------------------

def _emit(nc, tc, ctx, T, H, E, reps=1):
    import contextlib
    import concourse.mybir as mybir

    f32 = mybir.dt.float32
    f32r = mybir.dt.float32r
    bf16 = mybir.dt.bfloat16
    Alu = mybir.AluOpType
    Act = mybir.ActivationFunctionType
    X = mybir.AxisListType.X

    KC = H // 128          # contraction chunks over h; also h-out tiles
    NT = T // 128          # token tiles
    NE = E // 128          # ext tiles
    WH = min(512, H)       # h_out free-dim chunk
    NH = H // WH
    WT = min(512, T)       # token free-dim chunk
    NTW = T // WT
    assert E <= 512, "s_ext PSUM group assumes E <= 512"

    xT_d = nc.declare_dram_parameter("xT", [H, T], f32, isOutput=False)
    eT_d = nc.declare_dram_parameter("eT", [H, E], f32, isOutput=False)
    c_d = nc.declare_dram_parameter("Cq", [H, H], f32, isOutput=False)
    d_d = nc.declare_dram_parameter("Dq", [H, H], f32, isOutput=False)
    wv_d = nc.declare_dram_parameter("Wv", [H, H], f32, isOutput=False)
    w2c_d = nc.declare_dram_parameter("w2c", [128, KC], f32, isOutput=False)
    bvb_d = nc.declare_dram_parameter("bvb", [128, H], f32, isOutput=False)
    onesc_d = nc.declare_dram_parameter("onesc", [128, 2], f32, isOutput=False)
    onesr_d = nc.declare_dram_parameter("onesr", [1, 128], f32, isOutput=False)
    eye_d = nc.declare_dram_parameter("eye", [128, 128], f32, isOutput=False)
    eyeb_d = nc.declare_dram_parameter("eyeb", [128, 128], bf16,
                                       isOutput=False)
    out_d = nc.declare_dram_parameter("out", [T, H], f32, isOutput=True)

    cp = ctx.enter_context(tc.tile_pool(name="cp", bufs=1))
    small = ctx.enter_context(tc.tile_pool(name="small", bufs=3))

    # ---- persistent SBUF tensors -----------------------------------------
    M = cp.tile([128, KC * E], f32r, name="M")      # C @ ext^T hidden-major
    Vx = cp.tile([128, NE * H], bf16, name="Vx")    # V_ext E-major (raw)
    xT = cp.tile([128, KC * T], f32r, name="xT")    # chunk kc: cols [kc*T,+T)
    eT = cp.tile([128, KC * E], f32r, name="eT")    # chunk kc: cols [kc*E,+E)
    Cq = cp.tile([128, KC * H], f32r, name="Cq")    # C chunks [i-part, j]
    Dq = cp.tile([128, KC * H], f32r, name="Dq")    # C^T chunks [i-part, j]
    Wv = cp.tile([128, KC * H], f32r, name="Wv")
    u = cp.tile([1, E], f32r, name="u")
    ss_col = cp.tile([128, 2 * NT], f32, name="ss_col")
    ss_row = cp.tile([2, T], f32, name="ss_row")
    w2c = cp.tile([128, KC], f32, name="w2c")
    w2r = cp.tile([128, KC], f32r, name="w2r")
    bvb = cp.tile([128, H], f32, name="bvb")
    ones_c = cp.tile([128, 2], f32r, name="ones_c")
    ones_r = cp.tile([1, 128], f32r, name="ones_r")
    ident = cp.tile([128, 128], f32r, name="ident")
    identb = cp.tile([128, 128], bf16, name="identb")

    for kc in range(KC):
        nc.sync.dma_start(xT[:, kc * T:(kc + 1) * T],
                          xT_d[kc * 128:(kc + 1) * 128, :].bitcast(f32r))
        nc.sync.dma_start(eT[:, kc * E:(kc + 1) * E],
                          eT_d[kc * 128:(kc + 1) * 128, :].bitcast(f32r))
        nc.sync.dma_start(Cq[:, kc * H:(kc + 1) * H],
                          c_d[kc * 128:(kc + 1) * 128, :].bitcast(f32r))
        nc.sync.dma_start(Dq[:, kc * H:(kc + 1) * H],
                          d_d[kc * 128:(kc + 1) * 128, :].bitcast(f32r))
        nc.scalar.dma_start(Wv[:, kc * H:(kc + 1) * H],
                            wv_d[kc * 128:(kc + 1) * 128, :].bitcast(f32r))
    nc.sync.dma_start(w2c[:], w2c_d[:])
    nc.sync.dma_start(w2r[:], w2c_d[:].bitcast(f32r))
    nc.sync.dma_start(bvb[:], bvb_d[:])
    nc.sync.dma_start(ones_c[:], onesc_d[:].bitcast(f32r))
    nc.sync.dma_start(ones_r[:], onesr_d[:].bitcast(f32r))
    nc.sync.dma_start(ident[:], eye_d[:].bitcast(f32r))
    nc.sync.dma_start(identb[:], eyeb_d[:])

    Eng = mybir.EngineType
    if reps <= 1:
        loop_cm = contextlib.nullcontext()
    elif _LOOP_MODE == "plain":
        loop_cm = tc.For_i(0, reps, 1)
    else:
        loop_cm = tc.For_i(0, reps, 1,
                           hint_engines=(Eng.PE, Eng.DVE, Eng.Activation,
                                         Eng.SP, Eng.Pool),
                           staggered_reset=True)
    with loop_cm:
        # ---- phase Y: psy = C^T @ xT per j-tile; s_self via d-mult -------
        with tc.tile_pool(name="ps_y", bufs=4, space="PSUM") as ps_y, \
             tc.tile_pool(name="ps_ss", bufs=1, space="PSUM") as ps_ss, \
             tc.tile_pool(name="work_k", bufs=2) as work_k:
            sself = ps_ss.tile([2, T], f32, name="sself")
            for j in range(KC):
                psy = [ps_y.tile([128, WT], f32, name="psy")
                       for _ in range(NTW)]
                for kc in range(KC):
                    wc = Cq[:, kc * H + j * 128: kc * H + (j + 1) * 128]
                    for n in range(NTW):
                        nc.tensor.matmul(
                            psy[n][:], wc,
                            xT[:, kc * T + n * WT: kc * T + (n + 1) * WT],
                            start=(kc == 0), stop=(kc == KC - 1))
                # d = (y + w2) * xT, straight from PSUM
                d = work_k.tile([128, T], f32r, name="d")
                for n in range(NTW):
                    nc.vector.scalar_tensor_tensor(
                        d[:, n * WT:(n + 1) * WT], psy[n][:],
                        w2c[:, j:j + 1],
                        xT[:, j * T + n * WT: j * T + (n + 1) * WT],
                        Alu.add, Alu.mult)
                for n in range(NTW):
                    nc.tensor.matmul(sself[:, n * WT:(n + 1) * WT],
                                     ones_c[:], d[:, n * WT:(n + 1) * WT],
                                     start=(j == 0), stop=(j == KC - 1))
            # split the 2-partition copy across DVE+ACT (slow: 2 lanes only)
            nc.vector.tensor_copy(ss_row[:, 0:WT], sself[:, 0:WT])
            nc.scalar.copy(ss_row[:, WT:T], sself[:, WT:T])

        # ---- phase M: M = D^T @ eT (hidden-major), u = w2^T @ eT ---------
        with tc.tile_pool(name="ps_m", bufs=4, space="PSUM") as ps_m, \
             tc.tile_pool(name="ps_u", bufs=1, space="PSUM") as ps_u:
            psu = ps_u.tile([1, E], f32, name="psu")
            for kc in range(KC):
                nc.tensor.matmul(psu[:], w2r[:, kc:kc + 1],
                                 eT[:, kc * E:(kc + 1) * E],
                                 start=(kc == 0), stop=(kc == KC - 1))
            nc.vector.tensor_copy(u[:], psu[:])
            for j in range(KC):
                psm = ps_m.tile([128, E], f32, name="psm")
                for kc in range(KC):
                    nc.tensor.matmul(
                        psm[:],
                        Dq[:, kc * H + j * 128: kc * H + (j + 1) * 128],
                        eT[:, kc * E:(kc + 1) * E],
                        start=(kc == 0), stop=(kc == KC - 1))
                nc.scalar.copy(M[:, j * E:(j + 1) * E], psm[:])

        # ---- phase V_ext: Vx = (eT)^T @ Wv (raw), E-major ----------------
        # (the s_self transposes are interleaved so their DVE/ACT inputs
        # overlap the V_ext matmul stream instead of blocking the PE)
        with tc.tile_pool(name="ps_ve", bufs=4, space="PSUM") as ps_ve, \
             tc.tile_pool(name="ps_tr0", bufs=2, space="PSUM") as ps_tr0:
            for eo in range(NE):
                psv = [ps_ve.tile([128, WH], f32, name="psv")
                       for _ in range(NH)]
                for kc in range(KC):
                    lhsT = eT[:, kc * E + eo * 128: kc * E + (eo + 1) * 128]
                    for n in range(NH):
                        nc.tensor.matmul(
                            psv[n][:], lhsT,
                            Wv[:, kc * H + n * WH: kc * H + (n + 1) * WH],
                            start=(kc == 0), stop=(kc == KC - 1))
                for n in range(NH):
                    nc.scalar.copy(
                        Vx[:, eo * H + n * WH: eo * H + (n + 1) * WH],
                        psv[n][:])
                # two s_self transposes per eo tile
                for m in range(2 * eo, 2 * eo + 2):
                    pst2 = ps_tr0.tile([128, 2], f32, name="pst2")
                    nc.tensor.transpose(pst2[:],
                                        ss_row[:, m * 128:(m + 1) * 128],
                                        ident[0:2, 0:2].bitcast(f32))
                    nc.scalar.copy(ss_col[:, 2 * m:2 * m + 2], pst2[:])

        # ---- attention per token tile ------------------------------------
        with tc.tile_pool(name="ps_s", bufs=2, space="PSUM") as ps_att, \
             tc.tile_pool(name="ps_v", bufs=2, space="PSUM") as ps_v, \
             tc.tile_pool(name="ps_c", bufs=2, space="PSUM") as ps_c, \
             tc.tile_pool(name="ps_tr", bufs=2, space="PSUM") as ps_tr, \
             tc.tile_pool(name="work_a", bufs=2) as work_a, \
             tc.tile_pool(name="work_o", bufs=1) as work_o:
            for m in range(NT):
                # s_ext = xT_tile^T @ M + 1 x u  -> [128 tokens, E]
                ps_s = ps_att.tile([128, E], f32, name="ps_s")
                for kc in range(KC):
                    nc.tensor.matmul(
                        ps_s[:],
                        xT[:, kc * T + m * 128: kc * T + (m + 1) * 128],
                        M[:, kc * E:(kc + 1) * E],
                        start=(kc == 0), stop=False)
                nc.tensor.matmul(ps_s[:], ones_r[:], u[:],
                                 start=False, stop=True)

                # vtok = xT_tile.T @ Wv -> PSUM (raw v_tok, no softmax dep)
                psvt = [ps_v.tile([128, WH], f32, name="psvt")
                        for _ in range(NH)]
                for kc in range(KC):
                    lhsT = xT[:, kc * T + m * 128: kc * T + (m + 1) * 128]
                    for n in range(NH):
                        nc.tensor.matmul(
                            psvt[n][:], lhsT,
                            Wv[:, kc * H + n * WH: kc * H + (n + 1) * WH],
                            start=(kc == 0), stop=(kc == KC - 1))

                nmx = small.tile([128, 1], f32, name="nmx")
                nc.vector.tensor_reduce(nmx[:], ps_s[:], axis=X, op=Alu.max,
                                        negate=True)
                nself = small.tile([128, 1], f32, name="nself")
                nc.vector.tensor_scalar_mul(nself[:],
                                            ss_col[:, 2 * m:2 * m + 1], -1.0)
                nmx2 = small.tile([128, 1], f32, name="nmx2")
                nc.vector.tensor_tensor(nmx2[:], nmx[:], nself[:], Alu.min)

                # unnormalized probs (bf16); Z accumulated on the fly (f32)
                pe = work_a.tile([128, E], bf16, name="pe")
                Ze = small.tile([128, 1], f32, name="Ze")
                nc.scalar.activation(pe[:], ps_s[:], Act.Exp, bias=nmx2[:],
                                     scale=1.0, accum_out=Ze[:])
                p_self = small.tile([128, 1], f32, name="p_self")
                nc.scalar.activation(p_self[:], ss_col[:, 2 * m:2 * m + 1],
                                     Act.Exp, bias=nmx2[:], scale=1.0)
                Zt = small.tile([128, 1], f32, name="Zt")
                nc.vector.tensor_tensor(Zt[:], Ze[:], p_self[:], Alu.add)
                r = small.tile([128, 1], f32, name="r")
                nc.vector.reciprocal(r[:], Zt[:])
                a = small.tile([128, 1], f32, name="a")
                nc.vector.tensor_tensor(a[:], p_self[:], r[:], Alu.mult)

                # transpose unnormalized ext probs -> Pt (E-major, bf16)
                Pt = work_a.tile([128, NE * 128], bf16, name="Pt")
                for ec in range(NE):
                    pst = ps_tr.tile([128, 128], bf16, name="pst")
                    nc.tensor.transpose(pst[:],
                                        pe[:, ec * 128:(ec + 1) * 128],
                                        identb[:])
                    if ec % 2 == 0:
                        nc.scalar.copy(Pt[:, ec * 128:(ec + 1) * 128], pst[:])
                    else:
                        nc.vector.tensor_copy(
                            Pt[:, ec * 128:(ec + 1) * 128], pst[:])

                # ctx_ext = Pt.T @ Vx (unnormalized)
                psc = [ps_c.tile([128, WH], f32, name="psc")
                       for _ in range(NH)]
                for ec in range(NE):
                    lhsT = Pt[:, ec * 128:(ec + 1) * 128]
                    for n in range(NH):
                        nc.tensor.matmul(
                            psc[n][:], lhsT,
                            Vx[:, ec * H + n * WH: ec * H + (n + 1) * WH],
                            start=(ec == 0), stop=(ec == NE - 1))

                # out = psc*r + vtok*(p_self*r) + bvb, two fused STT passes
                out_sb = work_o.tile([128, H], f32, name="out_sb")
                for n in range(NH):
                    nc.vector.scalar_tensor_tensor(
                        out_sb[:, n * WH:(n + 1) * WH], psvt[n][:], a[:],
                        bvb[:, n * WH:(n + 1) * WH], Alu.mult, Alu.add)
                    nc.vector.scalar_tensor_tensor(
                        out_sb[:, n * WH:(n + 1) * WH], psc[n][:], r[:],
                        out_sb[:, n * WH:(n + 1) * WH], Alu.mult, Alu.add)
                nc.scalar.dma_start(out_d[m * 128:(m + 1) * 128, :], out_sb[:])


def _build_module(T, H, E, reps=1):
    from contextlib import ExitStack
    import concourse.tile as tile
    from concourse import bacc

    nc = bacc.Bacc(None)
    with ExitStack() as ctx:
        tc = ctx.enter_context(tile.TileContext(nc))
        _emit(nc, tc, ctx, T, H, E, reps)
    nc.finalize()
    return nc


# --------------------------------------------------------------------------
# host side
# --------------------------------------------------------------------------

def _shard_inputs(hidden_states, external_embeddings, Wq, bq, Wk, bk, Wv, bv):
    """Build the per-core input maps (host-side layout prep).

    bk cancels in the softmax (it adds q.bk to every score of a token),
    so the score pipeline only needs C = Wq @ Wk^T and w2 = Wk @ bq.
    """
    import ml_dtypes

    hs = np.asarray(hidden_states, dtype=np.float32)
    ext = np.asarray(external_embeddings, dtype=np.float32)
    Wq64 = np.asarray(Wq, dtype=np.float64)
    Wk64 = np.asarray(Wk, dtype=np.float64)
    Wv = np.ascontiguousarray(np.asarray(Wv, dtype=np.float32))
    bq = np.asarray(bq, dtype=np.float64)
    bv = np.asarray(bv, dtype=np.float32)

    C = np.ascontiguousarray((Wq64 @ Wk64.T).astype(np.float32))   # [H,H]
    D = np.ascontiguousarray(C.T)
    w2 = (Wk64 @ bq).astype(np.float32)                            # [H]

    KC = H // 128
    w2c = np.ascontiguousarray(w2.reshape(KC, 128).T)  # [128, KC]
    bvb = np.ascontiguousarray(np.broadcast_to(bv.reshape(1, H), (128, H)))
    eyeb = _EYE.astype(ml_dtypes.bfloat16)

    flat = hs.reshape(B * S, H)
    in_maps = []
    for c in range(NCORES):
        b = (c * T) // S
        xT = np.ascontiguousarray(flat[c * T:(c + 1) * T, :].T)  # [H, T]
        eT = np.ascontiguousarray(ext[b].T)                      # [H, E]
        in_maps.append({
            "xT": xT, "eT": eT,
            "Cq": C, "Dq": D, "Wv": Wv,
            "w2c": w2c, "bvb": bvb,
            "onesc": _ONESC, "onesr": _ONESR, "eye": _EYE, "eyeb": eyeb,
        })
    return in_maps


def kernel(hidden_states, external_embeddings, Wq, bq, Wk, bk, Wv, bv):
    from concourse.bass_utils import run_bass_kernel_spmd

    key = "main"
    if key not in _RUNNER_CACHE:
        _RUNNER_CACHE[key] = _build_module(T, H, E)
    nc = _RUNNER_CACHE[key]

    in_maps = _shard_inputs(hidden_states, external_embeddings,
                            Wq, bq, Wk, bk, Wv, bv)
    res = run_bass_kernel_spmd(nc, in_maps, list(range(NCORES)))
    out = np.concatenate([res.results[c]["out"] for c in range(NCORES)],
                         axis=0)
    return out.reshape(B, S, H)


# revision 30
# speedup vs baseline: 1.9719x; 1.0043x over previous
"""Trainium2 Bass kernel for ExternalEmbeddingSelfAttention.

Reference computation (per batch b):
    q     = hs @ Wq + bq           [S,H]
    k_tok = hs @ Wk + bk           [S,H]
    v_tok = hs @ Wv + bv           [S,H]
    k_ext = ext @ Wk + bk          [E,H]
    v_ext = ext @ Wv + bv          [E,H]
    s_self[t] = q[t] . k_tok[t]
    s_ext = q @ k_ext^T            [S,E]
    probs = softmax([s_self, s_ext], axis=-1)  (no 1/sqrt(d) scaling)
    out   = probs[:,0:1]*v_tok + probs[:,1:] @ v_ext

Score reformulation (v6).  Softmax is invariant to adding a constant to a
token's whole score row; q.bk appears in EVERY score of token t (self and
ext), so bk drops out entirely.  What remains factors through the
weight-only matrix C = Wq @ Wk^T and vector w2 = Wk @ bq (both computed
once on the host):
    s_ext[t,e] = x_t.C.ext_e + w2.ext_e   = (xT^T M)[t,e] + u[e]
                 with M = C @ ext^T [H,E],  u = w2^T ext^T [1,E]
    s_self[t]  = x_t.C.x_t + w2.x_t       = sum_h xT[h,t]*y[h,t]
                 with y = C^T x + w2 (w2 folded in as the eviction bias)
This deletes the Q and K_ext projections outright: M costs half a
projection (E=512 vs T=1024 columns) and everything else reuses existing
streams.  Per-core PE work drops ~17% vs the direct formulation.

Sharding: 8 cores, each takes 1024 contiguous tokens of the flattened
(B*S, H) token axis (core i -> batch i//2, S-half i%2).  Each core also
computes its batch's external M/u/V_ext (duplicated across the 2 cores
sharing a batch).

Device algorithm (per core, T=1024 tokens):
  - All operands live in SBUF persistently (xT, eT, C, D=C^T, Wv, biases);
    the steady-state loop has NO weight DMA, only the output stream.
  - y-proj: psy = C^T @ xT per h-tile; d = (psy + w2)*xT on DVE straight
    from PSUM; s_self accumulated [2,T] on PE via ones^T @ d; transposed
    to per-token columns during the V_ext phase.
  - M = D^T @ eT  (hidden-major [H,E], fp32r), u = w2^T @ eT [1,E]
  - Vx = ext @ Wv (raw, no bias)  E-major [E,H] bf16
  - Attention per 128-token tile:
      s_ext = xT_tile^T @ M (+ ones^T x u row)  (PE, fp32r)
      vtok  = xT_tile^T @ Wv -> PSUM            (PE, no softmax dep)
      softmax w/ self column folded in          (DVE max / ACT exp -> bf16)
      Pt    = transpose(exp probs)              (PE, bf16)
      ctx   = Pt^T @ Vx -> PSUM                 (PE, bf16, fast wt load)
      out   = psc*r + vtok*(p_self*r) + bvb     (DVE, 2 fused STT passes)

Precision: the score pipeline is fp32/fp32r end to end (C is formed in
f64 on the host, rounded to f32).  Only the value path (probs, Vx) is
bf16; probs and values are O(1) so the 2^-9 relative error lands well
inside the 2e-2 gate.
"""

import numpy as np

B, S, E, H = 4, 2048, 512, 1024
NCORES = 8
T = (B * S) // NCORES  # tokens per core = 1024

_RUNNER_CACHE = {}

# experiment knob (probes may flip; default is the shipping configuration)
_LOOP_MODE = "staggered"   # "staggered" | "plain"

_ONESC = np.ones((128, 2), dtype=np.float32)
_ONESR = np.ones((1, 128), dtype=np.float32)
_EYE = np.eye(128, dtype=np.float32)


# --------------------------------------------------------------------------
# device kernel emission
# --------------------------------------------------------------------------

def _emit(nc, tc, ctx, T, H, E, reps=1):
    import contextlib
    import concourse.mybir as mybir

    f32 = mybir.dt.float32
    f32r = mybir.dt.float32r
    bf16 = mybir.dt.bfloat16
    Alu = mybir.AluOpType
    Act = mybir.ActivationFunctionType
    X = mybir.AxisListType.X

    KC = H // 128          # contraction chunks over h; also h-out tiles
    NT = T // 128          # token tiles
    NE = E // 128          # ext tiles
    WH = min(512, H)       # h_out free-dim chunk
    NH = H // WH
    WT = min(512, T)       # token free-dim chunk
    NTW = T // WT
    assert E <= 512, "s_ext PSUM group assumes E <= 512"

    xT_d = nc.declare_dram_parameter("xT", [H, T], f32, isOutput=False)
    eT_d = nc.declare_dram_parameter("eT", [H, E], f32, isOutput=False)
    c_d = nc.declare_dram_parameter("Cq", [H, H], f32, isOutput=False)
    d_d = nc.declare_dram_parameter("Dq", [H, H], f32, isOutput=False)
    wv_d = nc.declare_dram_parameter("Wv", [H, H], f32, isOutput=False)
    w2c_d = nc.declare_dram_parameter("w2c", [128, KC], f32, isOutput=False)
    bvb_d = nc.declare_dram_parameter("bvb", [128, H], f32, isOutput=False)
    onesc_d = nc.declare_dram_parameter("onesc", [128, 2], f32, isOutput=False)
    onesr_d = nc.declare_dram_parameter("onesr", [1, 128], f32, isOutput=False)
    eye_d = nc.declare_dram_parameter("eye", [128, 128], f32, isOutput=False)
    eyeb_d = nc.declare_dram_parameter("eyeb", [128, 128], bf16,
                                       isOutput=False)
    out_d = nc.declare_dram_parameter("out", [T, H], f32, isOutput=True)

    cp = ctx.enter_context(tc.tile_pool(name="cp", bufs=1))
    small = ctx.enter_context(tc.tile_pool(name="small", bufs=3))

    # ---- persistent SBUF tensors -----------------------------------------
    M = cp.tile([128, KC * E], f32r, name="M")      # C @ ext^T hidden-major
    Vx = cp.tile([128, NE * H], bf16, name="Vx")    # V_ext E-major (raw)
    xT = cp.tile([128, KC * T], f32r, name="xT")    # chunk kc: cols [kc*T,+T)
    eT = cp.tile([128, KC * E], f32r, name="eT")    # chunk kc: cols [kc*E,+E)
    Cq = cp.tile([128, KC * H], f32r, name="Cq")    # C chunks [i-part, j]
    Dq = cp.tile([128, KC * H], f32r, name="Dq")    # C^T chunks [i-part, j]
    Wv = cp.tile([128, KC * H], f32r, name="Wv")
    u = cp.tile([1, E], f32r, name="u")
    ss_col = cp.tile([128, 2 * NT], f32, name="ss_col")
    ss_row = cp.tile([2, T], f32, name="ss_row")
    w2c = cp.tile([128, KC], f32, name="w2c")
    w2r = cp.tile([128, KC], f32r, name="w2r")
    bvb = cp.tile([128, H], f32, name="bvb")
    ones_c = cp.tile([128, 2], f32r, name="ones_c")
    ones_r = cp.tile([1, 128], f32r, name="ones_r")
    ident = cp.tile([128, 128], f32r, name="ident")
    identb = cp.tile([128, 128], bf16, name="identb")

    for kc in range(KC):
        nc.sync.dma_start(xT[:, kc * T:(kc + 1) * T],
                          xT_d[kc * 128:(kc + 1) * 128, :].bitcast(f32r))
        nc.sync.dma_start(eT[:, kc * E:(kc + 1) * E],
                          eT_d[kc * 128:(kc + 1) * 128, :].bitcast(f32r))
        nc.sync.dma_start(Cq[:, kc * H:(kc + 1) * H],
                          c_d[kc * 128:(kc + 1) * 128, :].bitcast(f32r))
        nc.sync.dma_start(Dq[:, kc * H:(kc + 1) * H],
                          d_d[kc * 128:(kc + 1) * 128, :].bitcast(f32r))
        nc.scalar.dma_start(Wv[:, kc * H:(kc + 1) * H],
                            wv_d[kc * 128:(kc + 1) * 128, :].bitcast(f32r))
    nc.sync.dma_start(w2c[:], w2c_d[:])
    nc.sync.dma_start(w2r[:], w2c_d[:].bitcast(f32r))
    nc.sync.dma_start(bvb[:], bvb_d[:])
    nc.sync.dma_start(ones_c[:], onesc_d[:].bitcast(f32r))
    nc.sync.dma_start(ones_r[:], onesr_d[:].bitcast(f32r))
    nc.sync.dma_start(ident[:], eye_d[:].bitcast(f32r))
    nc.sync.dma_start(identb[:], eyeb_d[:])

    Eng = mybir.EngineType
    if reps <= 1:
        loop_cm = contextlib.nullcontext()
    elif _LOOP_MODE == "plain":
        loop_cm = tc.For_i(0, reps, 1)
    else:
        loop_cm = tc.For_i(0, reps, 1,
                           hint_engines=(Eng.PE, Eng.DVE, Eng.Activation,
                                         Eng.SP, Eng.Pool),
                           staggered_reset=True)
    with loop_cm:
        # ---- phase Y: psy = C^T @ xT per j-tile; s_self via d-mult -------
        with tc.tile_pool(name="ps_y", bufs=4, space="PSUM") as ps_y, \
             tc.tile_pool(name="ps_ss", bufs=1, space="PSUM") as ps_ss, \
             tc.tile_pool(name="work_k", bufs=2) as work_k:
            sself = ps_ss.tile([2, T], f32, name="sself")
            for j in range(KC):
                psy = [ps_y.tile([128, WT], f32, name="psy")
                       for _ in range(NTW)]
                for kc in range(KC):
                    wc = Cq[:, kc * H + j * 128: kc * H + (j + 1) * 128]
                    for n in range(NTW):
                        nc.tensor.matmul(
                            psy[n][:], wc,
                            xT[:, kc * T + n * WT: kc * T + (n + 1) * WT],
                            start=(kc == 0), stop=(kc == KC - 1))
                # d = (y + w2) * xT, straight from PSUM
                d = work_k.tile([128, T], f32r, name="d")
                for n in range(NTW):
                    nc.vector.scalar_tensor_tensor(
                        d[:, n * WT:(n + 1) * WT], psy[n][:],
                        w2c[:, j:j + 1],
                        xT[:, j * T + n * WT: j * T + (n + 1) * WT],
                        Alu.add, Alu.mult)
                for n in range(NTW):
                    nc.tensor.matmul(sself[:, n * WT:(n + 1) * WT],
                                     ones_c[:], d[:, n * WT:(n + 1) * WT],
                                     start=(j == 0), stop=(j == KC - 1))
            # split the 2-partition copy across DVE+ACT (slow: 2 lanes only)
            nc.vector.tensor_copy(ss_row[:, 0:WT], sself[:, 0:WT])
            nc.scalar.copy(ss_row[:, WT:T], sself[:, WT:T])

        # ---- phase M: M = D^T @ eT (hidden-major), u = w2^T @ eT ---------
        with tc.tile_pool(name="ps_m", bufs=4, space="PSUM") as ps_m, \
             tc.tile_pool(name="ps_u", bufs=1, space="PSUM") as ps_u:
            psu = ps_u.tile([1, E], f32, name="psu")
            for kc in range(KC):
                nc.tensor.matmul(psu[:], w2r[:, kc:kc + 1],
                                 eT[:, kc * E:(kc + 1) * E],
                                 start=(kc == 0), stop=(kc == KC - 1))
            nc.vector.tensor_copy(u[:], psu[:])
            for j in range(KC):
                psm = ps_m.tile([128, E], f32, name="psm")
                for kc in range(KC):
                    nc.tensor.matmul(
                        psm[:],
                        Dq[:, kc * H + j * 128: kc * H + (j + 1) * 128],
                        eT[:, kc * E:(kc + 1) * E],
                        start=(kc == 0), stop=(kc == KC - 1))
                nc.scalar.copy(M[:, j * E:(j + 1) * E], psm[:])

        # ---- phase V_ext: Vx = (eT)^T @ Wv (raw), E-major ----------------
        # (the s_self transposes are interleaved so their DVE/ACT inputs
        # overlap the V_ext matmul stream instead of blocking the PE)
        with tc.tile_pool(name="ps_ve", bufs=4, space="PSUM") as ps_ve, \
             tc.tile_pool(name="ps_tr0", bufs=2, space="PSUM") as ps_tr0:
            for eo in range(NE):
                psv = [ps_ve.tile([128, WH], f32, name="psv")
                       for _ in range(NH)]
                for kc in range(KC):
                    lhsT = eT[:, kc * E + eo * 128: kc * E + (eo + 1) * 128]
                    for n in range(NH):
                        nc.tensor.matmul(
                            psv[n][:], lhsT,
                            Wv[:, kc * H + n * WH: kc * H + (n + 1) * WH],
                            start=(kc == 0), stop=(kc == KC - 1))
                for n in range(NH):
                    nc.scalar.copy(
                        Vx[:, eo * H + n * WH: eo * H + (n + 1) * WH],
                        psv[n][:])
                # two s_self transposes per eo tile
                for m in range(2 * eo, 2 * eo + 2):
                    pst2 = ps_tr0.tile([128, 2], f32, name="pst2")
                    nc.tensor.transpose(pst2[:],
                                        ss_row[:, m * 128:(m + 1) * 128],
                                        ident[0:2, 0:2].bitcast(f32))
                    nc.scalar.copy(ss_col[:, 2 * m:2 * m + 2], pst2[:])

        # ---- attention per token tile ------------------------------------
        with tc.tile_pool(name="ps_s", bufs=2, space="PSUM") as ps_att, \
             tc.tile_pool(name="ps_v", bufs=2, space="PSUM") as ps_v, \
             tc.tile_pool(name="ps_c", bufs=2, space="PSUM") as ps_c, \
             tc.tile_pool(name="ps_tr", bufs=2, space="PSUM") as ps_tr, \
             tc.tile_pool(name="work_a", bufs=2) as work_a, \
             tc.tile_pool(name="work_o", bufs=1) as work_o:
            for m in range(NT):
                # s_ext = xT_tile^T @ M + 1 x u  -> [128 tokens, E]
                ps_s = ps_att.tile([128, E], f32, name="ps_s")
                for kc in range(KC):
                    nc.tensor.matmul(
                        ps_s[:],
                        xT[:, kc * T + m * 128: kc * T + (m + 1) * 128],
                        M[:, kc * E:(kc + 1) * E],
                        start=(kc == 0), stop=False)
                nc.tensor.matmul(ps_s[:], ones_r[:], u[:],
                                 start=False, stop=True)

                # vtok = xT_tile.T @ Wv -> PSUM (raw v_tok, no softmax dep)
                psvt = [ps_v.tile([128, WH], f32, name="psvt")
                        for _ in range(NH)]
                for kc in range(KC):
                    lhsT = xT[:, kc * T + m * 128: kc * T + (m + 1) * 128]
                    for n in range(NH):
                        nc.tensor.matmul(
                            psvt[n][:], lhsT,
                            Wv[:, kc * H + n * WH: kc * H + (n + 1) * WH],
                            start=(kc == 0), stop=(kc == KC - 1))

                nmx = small.tile([128, 1], f32, name="nmx")
                nc.vector.tensor_reduce(nmx[:], ps_s[:], axis=X, op=Alu.max,
                                        negate=True)
                nself = small.tile([128, 1], f32, name="nself")
                nc.vector.tensor_scalar_mul(nself[:],
                                            ss_col[:, 2 * m:2 * m + 1], -1.0)
                nmx2 = small.tile([128, 1], f32, name="nmx2")
                nc.vector.tensor_tensor(nmx2[:], nmx[:], nself[:], Alu.min)

                # unnormalized probs (bf16); Z accumulated on the fly (f32)
                pe = work_a.tile([128, E], bf16, name="pe")
                Ze = small.tile([128, 1], f32, name="Ze")
                nc.scalar.activation(pe[:], ps_s[:], Act.Exp, bias=nmx2[:],
                                     scale=1.0, accum_out=Ze[:])
                p_self = small.tile([128, 1], f32, name="p_self")
                nc.scalar.activation(p_self[:], ss_col[:, 2 * m:2 * m + 1],
                                     Act.Exp, bias=nmx2[:], scale=1.0)
                Zt = small.tile([128, 1], f32, name="Zt")
                nc.vector.tensor_tensor(Zt[:], Ze[:], p_self[:], Alu.add)
                r = small.tile([128, 1], f32, name="r")
                nc.vector.reciprocal(r[:], Zt[:])
                a = small.tile([128, 1], f32, name="a")
                nc.vector.tensor_tensor(a[:], p_self[:], r[:], Alu.mult)

                # transpose unnormalized ext probs -> Pt (E-major, bf16)
                Pt = work_a.tile([128, NE * 128], bf16, name="Pt")
                for ec in range(NE):
                    pst = ps_tr.tile([128, 128], bf16, name="pst")
                    nc.tensor.transpose(pst[:],
                                        pe[:, ec * 128:(ec + 1) * 128],
                                        identb[:])
                    if ec % 2 == 0:
                        nc.scalar.copy(Pt[:, ec * 128:(ec + 1) * 128], pst[:])
                    else:
                        nc.vector.tensor_copy(
                            Pt[:, ec * 128:(ec + 1) * 128], pst[:])

                # ctx_ext = Pt.T @ Vx (unnormalized)
                psc = [ps_c.tile([128, WH], f32, name="psc")
                       for _ in range(NH)]
                for ec in range(NE):
                    lhsT = Pt[:, ec * 128:(ec + 1) * 128]
                    for n in range(NH):
                        nc.tensor.matmul(
                            psc[n][:], lhsT,
                            Vx[:, ec * H + n * WH: ec * H + (n + 1) * WH],
                            start=(ec == 0), stop=(ec == NE - 1))

                # out = psc*r + vtok*(p_self*r) + bvb, two fused STT passes
                out_sb = work_o.tile([128, H], f32, name="out_sb")
                for n in range(NH):
                    nc.vector.scalar_tensor_tensor(
                        out_sb[:, n * WH:(n + 1) * WH], psvt[n][:], a[:],
                        bvb[:, n * WH:(n + 1) * WH], Alu.mult, Alu.add)
                    nc.vector.scalar_tensor_tensor(
                        out_sb[:, n * WH:(n + 1) * WH], psc[n][:], r[:],
                        out_sb[:, n * WH:(n + 1) * WH], Alu.mult, Alu.add)
                nc.scalar.dma_start(out_d[m * 128:(m + 1) * 128, :], out_sb[:])


def _build_module(T, H, E, reps=1):
    from contextlib import ExitStack
    import concourse.tile as tile
    from concourse import bacc

    nc = bacc.Bacc(None)
    with ExitStack() as ctx:
        tc = ctx.enter_context(tile.TileContext(nc))
        _emit(nc, tc, ctx, T, H, E, reps)
    nc.finalize()
    return nc


# --------------------------------------------------------------------------
# host side
# --------------------------------------------------------------------------

def _shard_inputs(hidden_states, external_embeddings, Wq, bq, Wk, bk, Wv, bv):
    """Build the per-core input maps (host-side layout prep).

    bk cancels in the softmax (it adds q.bk to every score of a token),
    so the score pipeline only needs C = Wq @ Wk^T and w2 = Wk @ bq.
    """
    import ml_dtypes

    hs = np.asarray(hidden_states, dtype=np.float32)
    ext = np.asarray(external_embeddings, dtype=np.float32)
    Wq64 = np.asarray(Wq, dtype=np.float64)
    Wk64 = np.asarray(Wk, dtype=np.float64)
    Wv = np.ascontiguousarray(np.asarray(Wv, dtype=np.float32))
    bq = np.asarray(bq, dtype=np.float64)
    bv = np.asarray(bv, dtype=np.float32)

    C = np.ascontiguousarray((Wq64 @ Wk64.T).astype(np.float32))   # [H,H]
    D = np.ascontiguousarray(C.T)
    w2 = (Wk64 @ bq).astype(np.float32)                            # [H]

    KC = H // 128
    w2c = np.ascontiguousarray(w2.reshape(KC, 128).T)  # [128, KC]
    bvb = np.ascontiguousarray(np.broadcast_to(bv.reshape(1, H), (128, H)))
    eyeb = _EYE.astype(ml_dtypes.bfloat16)

    flat = hs.reshape(B * S, H)
    in_maps = []
    for c in range(NCORES):
        b = (c * T) // S
        xT = np.ascontiguousarray(flat[c * T:(c + 1) * T, :].T)  # [H, T]
        eT = np.ascontiguousarray(ext[b].T)                      # [H, E]
        in_maps.append({
            "xT": xT, "eT": eT,
            "Cq": C, "Dq": D, "Wv": Wv,
            "w2c": w2c, "bvb": bvb,
            "onesc": _ONESC, "onesr": _ONESR, "eye": _EYE, "eyeb": eyeb,
        })
    return in_maps


def kernel(hidden_states, external_embeddings, Wq, bq, Wk, bk, Wv, bv):
    from concourse.bass_utils import run_bass_kernel_spmd

    key = "main"
    if key not in _RUNNER_CACHE:
        _RUNNER_CACHE[key] = _build_module(T, H, E)
    nc = _RUNNER_CACHE[key]

    in_maps = _shard_inputs(hidden_states, external_embeddings,
                            Wq, bq, Wk, bk, Wv, bv)
    res = run_bass_kernel_spmd(nc, in_maps, list(range(NCORES)))
    out = np.concatenate([res.results[c]["out"] for c in range(NCORES)],
                         axis=0)
    return out.reshape(B, S, H)
